# revision 11
# baseline (speedup 1.0000x reference)
"""AdaTTSpUnit (16-expert MoE + 8 task gates + self-expert residual) on 8 TRN2
NeuronCores. Data-parallel over batch: each core computes 512 of the 4096 rows
end-to-end; no collectives.

Per-core math (B=512 rows, D=1024, H=512, O=512, T=8, E=2, TE=16):
  hT_e[h,b]   = relu(W1_e^T x^T + b1_e)        (PE, [h,b] layout - no transposes
  E_e[b,o]    = relu(hT_e^T W2_e + b2_e)        between layers)
  G[b,t,g]    = softmax_g(x gW + gb) + blockdiag(sew)
  out[b,t,o]  = sum_g G[b,t,g] * E_g[b,o]

Matmul operands are bf16 (f32 matmuls cost two PE passes); PSUM accumulation
is f32. The gate fusion is split by task: 'P' tasks run as diag(G-col) @ E
matmuls on the PE accumulating in PSUM (all E tiles are kept resident in
SBUF); 'A' tasks accumulate incrementally on the vector engine with
per-partition-scalar FMAs.
"""

import numpy as np

B, D, H, O = 4096, 1024, 512, 512
T, E = 8, 2
TE = T * E
NCORES = 8
BL = B // NCORES          # 512 rows per core
NBS = BL // 128           # 4 row-blocks of 128
ND = D // 128             # 8 contraction chunks for layer 1 / gates
NH = H // 128             # 4 contraction chunks for layer 2

# fusion path per task: 'P' = PE diag-matmul pass after the expert loop;
# 'A' = incremental DVE scalar_tensor_tensor
PATHS = ['P', 'P', 'P', 'P', 'A', 'A', 'A', 'A']

_CACHE = {}


def _build(zero_b1, zero_b2, zero_gb):
    import concourse.bass as bass
    import concourse.tile as tile
    from concourse import bacc, mybir
    from concourse.bass import ds
    from concourse.masks import make_identity

    f32 = mybir.dt.float32
    bf16 = mybir.dt.bfloat16
    nc = bacc.Bacc("TRN2", target_bir_lowering=False, debug=False,
                   num_devices=NCORES)

    x_ext = nc.dram_tensor("x", [BL, D], bf16, kind="ExternalInput").ap()
    w1_ext = nc.dram_tensor("eW1", [TE, D, H], bf16, kind="ExternalInput").ap()
    w2_ext = nc.dram_tensor("eW2", [TE, H, O], bf16, kind="ExternalInput").ap()
    gw_ext = nc.dram_tensor("gWf", [D, T * TE], bf16, kind="ExternalInput").ap()
    b1_ext = (None if zero_b1 else
              nc.dram_tensor("eb1T", [H, TE], f32, kind="ExternalInput").ap())
    b2_ext = (None if zero_b2 else
              nc.dram_tensor("eb2", [TE, O], bf16, kind="ExternalInput").ap())
    gb_ext = nc.dram_tensor("gbS", [2, T, TE], f32, kind="ExternalInput").ap()
    out_ext = nc.dram_tensor("out", [BL, T, O], bf16, kind="ExternalOutput").ap()

    with tile.TileContext(nc) as tc:
        with (
            tc.tile_pool(name="consts", bufs=1) as consts,
            tc.tile_pool(name="xin", bufs=2) as xin_pool,
            tc.tile_pool(name="w1", bufs=3) as w1_pool,
            tc.tile_pool(name="w2", bufs=3) as w2_pool,
            tc.tile_pool(name="ht", bufs=6) as ht_pool,
            tc.tile_pool(name="estore", bufs=1) as e_pool,
            tc.tile_pool(name="diag", bufs=8) as diag_pool,
            tc.tile_pool(name="ostage", bufs=4) as ostage_pool,
            tc.tile_pool(name="gsc", bufs=4) as gsc_pool,
            tc.tile_pool(name="acc", bufs=1) as acc_pool,
        ):
            identity = consts.tile([128, 128], bf16)
            make_identity(nc, identity[:])
            if not zero_b2:
                ones1 = consts.tile([1, 128], bf16)
                nc.vector.memset(ones1[:], 1.0)

            with (
                tc.tile_pool(name="pss", bufs=2, space="PSUM") as psum_small,
                tc.tile_pool(name="ps1", bufs=4, space="PSUM") as psum_l1,
                tc.tile_pool(name="ps2", bufs=2, space="PSUM") as psum_l2,
            ):
                # ---- x transpose: xT[d-part, dchunk, b] ----------------
                xT = consts.tile([128, ND, BL], bf16)
                for bs in range(NBS):
                    x_t = xin_pool.tile([128, D], bf16)
                    nc.sync.dma_start(out=x_t[:], in_=x_ext[ds(bs * 128, 128), :])
                    for dc in range(ND):
                        tp = psum_small.tile([128, 128], bf16, tag="pss",
                                             name="tp")
                        nc.tensor.transpose(tp[:], x_t[:, ds(dc * 128, 128)],
                                            identity[:])
                        nc.scalar.copy(xT[:, dc, ds(bs * 128, 128)], tp[:])

                # gate weights / bias consts (emitted after x so the
                # first expert's weight DMAs win the queue)
                gw_sb = consts.tile([128, ND, T * TE], bf16)
                for dc in range(ND):
                    nc.sync.dma_start(out=gw_sb[:, dc, :],
                                      in_=gw_ext[ds(dc * 128, 128), :])
                if not zero_b1:
                    b1_sb = consts.tile([128, NH, TE], f32)
                    for hc in range(NH):
                        nc.sync.dma_start(out=b1_sb[:, hc, :],
                                          in_=b1_ext[ds(hc * 128, 128), :])
                gbS_sb = consts.tile([128, 2, T, TE], f32)
                nc.sync.dma_start(
                    out=gbS_sb[:],
                    in_=bass.AP(tensor=gb_ext.tensor, offset=gb_ext.offset,
                                ap=[[0, 128]] + list(gb_ext.ap)),
                )
                gb_row = gbS_sb[:, 0]     # [128, T, TE]
                s_row = gbS_sb[:, 1]      # [128, T, TE]

                # ---- gates: G[b, t, g] per row-block -------------------
                G = []
                for bs in range(NBS):
                    lg = psum_small.tile([128, T * TE], f32, tag="pss",
                                         name="lg")
                    for dc in range(ND):
                        nc.tensor.matmul(
                            lg[:], xT[:, dc, ds(bs * 128, 128)], gw_sb[:, dc, :],
                            start=(dc == 0), stop=(dc == ND - 1))
                    logits = gsc_pool.tile([128, T, TE], f32, tag="logits")
                    lg3 = lg[:].rearrange("p (t g) -> p t g", t=T)
                    if zero_gb:
                        nc.scalar.copy(logits[:], lg3)
                    else:
                        nc.vector.scalar_tensor_tensor(
                            out=logits[:], in0=lg3, scalar=1.0, in1=gb_row,
                            op0=mybir.AluOpType.mult, op1=mybir.AluOpType.add)
                    nmx = gsc_pool.tile([128, T], f32, tag="nmx")
                    nc.vector.tensor_reduce(
                        out=nmx[:], in_=logits[:], axis=mybir.AxisListType.X,
                        op=mybir.AluOpType.max, negate=True)
                    ex = gsc_pool.tile([128, T, TE], f32, tag="ex")
                    for t in range(T):
                        nc.scalar.activation(
                            out=ex[:, t], in_=logits[:, t],
                            func=mybir.ActivationFunctionType.Exp,
                            bias=nmx[:, ds(t, 1)], scale=1.0)
                    sm = gsc_pool.tile([128, T], f32, tag="sm")
                    nc.vector.tensor_reduce(
                        out=sm[:], in_=ex[:], axis=mybir.AxisListType.X,
                        op=mybir.AluOpType.add)
                    rc = gsc_pool.tile([128, T], f32, tag="rc")
                    nc.vector.reciprocal(rc[:], sm[:])
                    g_bs = consts.tile([128, T, TE], f32, tag=f"g{bs}",
                                       name=f"g{bs}")
                    for t in range(T):
                        nc.vector.scalar_tensor_tensor(
                            out=g_bs[:, t], in0=ex[:, t],
                            scalar=rc[:, ds(t, 1)], in1=s_row[:, t],
                            op0=mybir.AluOpType.mult, op1=mybir.AluOpType.add)
                    G.append(g_bs)

                # ---- accumulators for 'A' tasks ------------------------
                acc = {t: acc_pool.tile([128, NBS, O], bf16, tag=f"acc{t}",
                                        name=f"acc{t}")
                       for t in range(T) if PATHS[t] == 'A'}

                # ---- expert loop: E[e][bs] resident in SBUF ------------
                Etiles = {}
                for e in range(TE):
                    w1t = w1_pool.tile([128, ND, H], bf16, tag="w1", name="w1t")
                    nc.sync.dma_start(
                        out=w1t[:],
                        in_=w1_ext[e].rearrange("(dc dp) h -> dp dc h", dp=128))
                    w2t = w2_pool.tile([128, NH, O], bf16, tag="w2", name="w2t")
                    nc.sync.dma_start(
                        out=w2t[:],
                        in_=w2_ext[e].rearrange("(hc hp) o -> hp hc o", hp=128))
                    if not zero_b2:
                        b2row = consts.tile([1, O], bf16, tag=f"b2_{e}",
                                            name=f"b2row{e}")
                        nc.sync.dma_start(out=b2row[:], in_=b2_ext[ds(e, 1), :])

                    ht = []
                    for hc in range(NH):
                        ps = psum_l1.tile([128, BL], f32, tag="ps1", name="ps1")
                        for dc in range(ND):
                            nc.tensor.matmul(
                                ps[:], w1t[:, dc, ds(hc * 128, 128)],
                                xT[:, dc, :],
                                start=(dc == 0), stop=(dc == ND - 1))
                        h_sb = ht_pool.tile([128, BL], bf16, tag="ht", name="ht")
                        nc.scalar.activation(
                            out=h_sb[:], in_=ps[:],
                            func=mybir.ActivationFunctionType.Relu,
                            bias=(0.0 if zero_b1 else b1_sb[:, hc, ds(e, 1)]),
                            scale=1.0)
                        ht.append(h_sb)

                    for bs in range(NBS):
                        ps2 = psum_l2.tile([128, O], f32, tag="ps2", name="ps2")
                        if not zero_b2:
                            nc.tensor.matmul(ps2[:], ones1[:], b2row[:],
                                             start=True, stop=False)
                        for hc in range(NH):
                            nc.tensor.matmul(
                                ps2[:], ht[hc][:, ds(bs * 128, 128)],
                                w2t[:, hc, :],
                                start=(zero_b2 and hc == 0), stop=(hc == NH - 1))
                        e_sb = e_pool.tile([128, O], bf16, tag=f"e{e}b{bs}",
                                           name=f"e{e}b{bs}")
                        nc.scalar.activation(
                            out=e_sb[:], in_=ps2[:],
                            func=mybir.ActivationFunctionType.Relu)
                        Etiles[(e, bs)] = e_sb
                        for t in range(T):
                            if PATHS[t] != 'A':
                                continue
                            if e == 0:
                                nc.vector.tensor_scalar(
                                    out=acc[t][:, bs], in0=e_sb[:],
                                    scalar1=G[bs][:, t, ds(e, 1)], scalar2=None,
                                    op0=mybir.AluOpType.mult)
                            else:
                                nc.vector.scalar_tensor_tensor(
                                    out=acc[t][:, bs], in0=e_sb[:],
                                    scalar=G[bs][:, t, ds(e, 1)],
                                    in1=acc[t][:, bs],
                                    op0=mybir.AluOpType.mult,
                                    op1=mybir.AluOpType.add)
                        if e == TE - 1:
                            for t in range(T):
                                if PATHS[t] == 'A':
                                    nc.sync.dma_start(
                                        out=out_ext[ds(bs * 128, 128), t, :],
                                        in_=acc[t][:, bs])

            # ---- fusion pass for 'P' tasks: diag(G-col) @ E in PSUM ----
            with tc.tile_pool(name="psf", bufs=6, space="PSUM") as psum_f:
                for bs in range(NBS):
                    for t in range(T):
                        if PATHS[t] != 'P':
                            continue
                        pf = psum_f.tile([128, O], f32, tag="psf", name="psf")
                        for e in range(TE):
                            dg = diag_pool.tile([128, 128], bf16, tag="diag",
                                                name="diag")
                            col = G[bs][:, t, ds(e, 1)]
                            colb = bass.AP(tensor=col.tensor, offset=col.offset,
                                           ap=[list(col.ap[0]), [0, 128]])
                            nc.gpsimd.tensor_tensor(
                                out=dg[:], in0=identity[:], in1=colb,
                                op=mybir.AluOpType.mult)
                            nc.tensor.matmul(
                                pf[:], dg[:], Etiles[(e, bs)][:],
                                start=(e == 0), stop=(e == TE - 1))
                        ostage = ostage_pool.tile([128, O], bf16, tag="ostage",
                                                  name="ostage")
                        nc.scalar.copy(ostage[:], pf[:])
                        nc.sync.dma_start(
                            out=out_ext[ds(bs * 128, 128), t, :],
                            in_=ostage[:])

    nc.compile()
    return nc


def _in_maps(inputs):
    import ml_dtypes

    bf = ml_dtypes.bfloat16
    x = np.ascontiguousarray(inputs["x"]).astype(bf)
    eW1 = np.ascontiguousarray(inputs["eW1"]).astype(bf)
    eW2 = np.ascontiguousarray(inputs["eW2"]).astype(bf)
    gWf = np.ascontiguousarray(
        np.asarray(inputs["gW"], np.float32).transpose(1, 0, 2)
        .reshape(D, T * TE)).astype(bf)
    eb1 = np.asarray(inputs["eb1"], np.float32)
    eb2 = np.asarray(inputs["eb2"], np.float32)
    gb = np.asarray(inputs["gb"], np.float32)
    sew = np.asarray(inputs["sew"], np.float32)
    S = np.zeros((T, TE), np.float32)
    for t in range(T):
        for e in range(E):
            S[t, t * E + e] = sew[t, e]
    gbS = np.ascontiguousarray(np.stack([gb, S]))

    zero_b1 = not np.any(eb1)
    zero_b2 = not np.any(eb2)
    zero_gb = not np.any(gb)
    common = dict(gWf=gWf, gbS=gbS, eW1=eW1, eW2=eW2)
    if not zero_b1:
        common["eb1T"] = np.ascontiguousarray(eb1.T)
    if not zero_b2:
        common["eb2"] = np.ascontiguousarray(eb2).astype(bf)
    shards = np.split(x, NCORES, axis=0)
    in_maps = [dict(common, x=np.ascontiguousarray(shards[i]))
               for i in range(NCORES)]
    return in_maps, (zero_b1, zero_b2, zero_gb)


def run_traced(trace=False, **inputs):
    from concourse.bass_utils import run_bass_kernel_spmd

    in_maps, key = _in_maps(inputs)
    if _CACHE.get("key") != key:
        _CACHE["nc"] = _build(*key)
        _CACHE["key"] = key
    res = run_bass_kernel_spmd(_CACHE["nc"], in_maps,
                               core_ids=list(range(NCORES)), trace=trace)
    out = np.concatenate([res.results[i]["out"] for i in range(NCORES)], axis=0)
    return np.asarray(out, dtype=np.float32), res


def kernel(**inputs):
    out, _ = run_traced(trace=False, **inputs)
    return out


# revision 12
# speedup vs baseline: 1.0862x; 1.0862x over previous
"""AdaTTSpUnit (16-expert MoE + 8 task gates + self-expert residual) on 8 TRN2
NeuronCores. Data-parallel over batch: each core computes 512 of the 4096 rows
end-to-end; no collectives.

Per-core math (B=512 rows, D=1024, H=512, O=512, T=8, E=2, TE=16):
  hT_e[h,b]   = relu(W1_e^T x^T + b1_e)        (PE, [h,b] layout - no transposes
  E_e[b,o]    = relu(hT_e^T W2_e + b2_e)        between layers)
  G[b,t,g]    = softmax_g(x gW + gb) + blockdiag(sew)
  out[b,t,o]  = sum_g G[b,t,g] * E_g[b,o]

Matmul operands are bf16 (f32 matmuls cost two PE passes); PSUM accumulation
is f32. The gate fusion is split by task: 'P' tasks run as diag(G-col) @ E
matmuls on the PE accumulating in PSUM (all E tiles are kept resident in
SBUF); 'A' tasks accumulate incrementally on the vector engine with
per-partition-scalar FMAs.
"""

import numpy as np

B, D, H, O = 4096, 1024, 512, 512
T, E = 8, 2
TE = T * E
NCORES = 8
BL = B // NCORES          # 512 rows per core
NBS = BL // 128           # 4 row-blocks of 128
ND = D // 128             # 8 contraction chunks for layer 1 / gates
NH = H // 128             # 4 contraction chunks for layer 2

# fusion path per task: 'P' = PE diag-matmul pass after the expert loop;
# 'A' = incremental DVE scalar_tensor_tensor
PATHS = ['P', 'P', 'P', 'P', 'P', 'A', 'A', 'A']

_CACHE = {}


def _build(zero_b1, zero_b2, zero_gb):
    import concourse.bass as bass
    import concourse.tile as tile
    from concourse import bacc, mybir
    from concourse.bass import ds
    from concourse.masks import make_identity

    f32 = mybir.dt.float32
    bf16 = mybir.dt.bfloat16
    nc = bacc.Bacc("TRN2", target_bir_lowering=False, debug=False,
                   num_devices=NCORES)

    x_ext = nc.dram_tensor("x", [BL, D], bf16, kind="ExternalInput").ap()
    w1_ext = nc.dram_tensor("eW1", [TE, D, H], bf16, kind="ExternalInput").ap()
    w2_ext = nc.dram_tensor("eW2", [TE, H, O], bf16, kind="ExternalInput").ap()
    gw_ext = nc.dram_tensor("gWf", [D, T * TE], bf16, kind="ExternalInput").ap()
    b1_ext = (None if zero_b1 else
              nc.dram_tensor("eb1T", [H, TE], f32, kind="ExternalInput").ap())
    b2_ext = (None if zero_b2 else
              nc.dram_tensor("eb2", [TE, O], bf16, kind="ExternalInput").ap())
    gb_ext = nc.dram_tensor("gbS", [2, T, TE], f32, kind="ExternalInput").ap()
    out_ext = nc.dram_tensor("out", [BL, T, O], bf16, kind="ExternalOutput").ap()

    with tile.TileContext(nc) as tc:
        with (
            tc.tile_pool(name="consts", bufs=1) as consts,
            tc.tile_pool(name="xin", bufs=2) as xin_pool,
            tc.tile_pool(name="w1", bufs=3) as w1_pool,
            tc.tile_pool(name="w2", bufs=3) as w2_pool,
            tc.tile_pool(name="ht", bufs=6) as ht_pool,
            tc.tile_pool(name="estore", bufs=1) as e_pool,
            tc.tile_pool(name="diag", bufs=8) as diag_pool,
            tc.tile_pool(name="ostage", bufs=4) as ostage_pool,
            tc.tile_pool(name="gsc", bufs=4) as gsc_pool,
            tc.tile_pool(name="acc", bufs=1) as acc_pool,
        ):
            identity = consts.tile([128, 128], bf16)
            make_identity(nc, identity[:])
            if not zero_b2:
                ones1 = consts.tile([1, 128], bf16)
                nc.vector.memset(ones1[:], 1.0)

            with (
                tc.tile_pool(name="pss", bufs=2, space="PSUM") as psum_small,
                tc.tile_pool(name="ps1", bufs=4, space="PSUM") as psum_l1,
                tc.tile_pool(name="ps2", bufs=2, space="PSUM") as psum_l2,
            ):
                # ---- x transpose: xT[d-part, dchunk, b] ----------------
                xT = consts.tile([128, ND, BL], bf16)
                for bs in range(NBS):
                    x_t = xin_pool.tile([128, D], bf16)
                    nc.sync.dma_start(out=x_t[:], in_=x_ext[ds(bs * 128, 128), :])
                    for dc in range(ND):
                        tp = psum_small.tile([128, 128], bf16, tag="pss",
                                             name="tp")
                        nc.tensor.transpose(tp[:], x_t[:, ds(dc * 128, 128)],
                                            identity[:])
                        nc.scalar.copy(xT[:, dc, ds(bs * 128, 128)], tp[:])

                # gate weights / bias consts (emitted after x so the
                # first expert's weight DMAs win the queue)
                gw_sb = consts.tile([128, ND, T * TE], bf16)
                for dc in range(ND):
                    nc.sync.dma_start(out=gw_sb[:, dc, :],
                                      in_=gw_ext[ds(dc * 128, 128), :])
                if not zero_b1:
                    b1_sb = consts.tile([128, NH, TE], f32)
                    for hc in range(NH):
                        nc.sync.dma_start(out=b1_sb[:, hc, :],
                                          in_=b1_ext[ds(hc * 128, 128), :])
                gbS_sb = consts.tile([128, 2, T, TE], f32)
                nc.sync.dma_start(
                    out=gbS_sb[:],
                    in_=bass.AP(tensor=gb_ext.tensor, offset=gb_ext.offset,
                                ap=[[0, 128]] + list(gb_ext.ap)),
                )
                gb_row = gbS_sb[:, 0]     # [128, T, TE]
                s_row = gbS_sb[:, 1]      # [128, T, TE]

                # ---- gates: G[b, t, g] per row-block -------------------
                G = []
                for bs in range(NBS):
                    lg = psum_small.tile([128, T * TE], f32, tag="pss",
                                         name="lg")
                    for dc in range(ND):
                        nc.tensor.matmul(
                            lg[:], xT[:, dc, ds(bs * 128, 128)], gw_sb[:, dc, :],
                            start=(dc == 0), stop=(dc == ND - 1))
                    logits = gsc_pool.tile([128, T, TE], f32, tag="logits")
                    lg3 = lg[:].rearrange("p (t g) -> p t g", t=T)
                    if zero_gb:
                        nc.scalar.copy(logits[:], lg3)
                    else:
                        nc.vector.scalar_tensor_tensor(
                            out=logits[:], in0=lg3, scalar=1.0, in1=gb_row,
                            op0=mybir.AluOpType.mult, op1=mybir.AluOpType.add)
                    nmx = gsc_pool.tile([128, T], f32, tag="nmx")
                    nc.vector.tensor_reduce(
                        out=nmx[:], in_=logits[:], axis=mybir.AxisListType.X,
                        op=mybir.AluOpType.max, negate=True)
                    ex = gsc_pool.tile([128, T, TE], f32, tag="ex")
                    for t in range(T):
                        nc.scalar.activation(
                            out=ex[:, t], in_=logits[:, t],
                            func=mybir.ActivationFunctionType.Exp,
                            bias=nmx[:, ds(t, 1)], scale=1.0)
                    sm = gsc_pool.tile([128, T], f32, tag="sm")
                    nc.vector.tensor_reduce(
                        out=sm[:], in_=ex[:], axis=mybir.AxisListType.X,
                        op=mybir.AluOpType.add)
                    rc = gsc_pool.tile([128, T], f32, tag="rc")
                    nc.vector.reciprocal(rc[:], sm[:])
                    g_bs = consts.tile([128, T, TE], f32, tag=f"g{bs}",
                                       name=f"g{bs}")
                    for t in range(T):
                        nc.vector.scalar_tensor_tensor(
                            out=g_bs[:, t], in0=ex[:, t],
                            scalar=rc[:, ds(t, 1)], in1=s_row[:, t],
                            op0=mybir.AluOpType.mult, op1=mybir.AluOpType.add)
                    G.append(g_bs)

                # ---- accumulators for 'A' tasks ------------------------
                acc = {t: acc_pool.tile([128, NBS, O], bf16, tag=f"acc{t}",
                                        name=f"acc{t}")
                       for t in range(T) if PATHS[t] == 'A'}

                # ---- expert loop: E[e][bs] resident in SBUF ------------
                Etiles = {}
                for e in range(TE):
                    w1t = w1_pool.tile([128, ND, H], bf16, tag="w1", name="w1t")
                    nc.sync.dma_start(
                        out=w1t[:],
                        in_=w1_ext[e].rearrange("(dc dp) h -> dp dc h", dp=128))
                    w2t = w2_pool.tile([128, NH, O], bf16, tag="w2", name="w2t")
                    nc.sync.dma_start(
                        out=w2t[:],
                        in_=w2_ext[e].rearrange("(hc hp) o -> hp hc o", hp=128))
                    if not zero_b2:
                        b2row = consts.tile([1, O], bf16, tag=f"b2_{e}",
                                            name=f"b2row{e}")
                        nc.sync.dma_start(out=b2row[:], in_=b2_ext[ds(e, 1), :])

                    ht = []
                    for hc in range(NH):
                        ps = psum_l1.tile([128, BL], f32, tag="ps1", name="ps1")
                        for dc in range(ND):
                            nc.tensor.matmul(
                                ps[:], w1t[:, dc, ds(hc * 128, 128)],
                                xT[:, dc, :],
                                start=(dc == 0), stop=(dc == ND - 1))
                        h_sb = ht_pool.tile([128, BL], bf16, tag="ht", name="ht")
                        nc.scalar.activation(
                            out=h_sb[:], in_=ps[:],
                            func=mybir.ActivationFunctionType.Relu,
                            bias=(0.0 if zero_b1 else b1_sb[:, hc, ds(e, 1)]),
                            scale=1.0)
                        ht.append(h_sb)

                    for bs in range(NBS):
                        ps2 = psum_l2.tile([128, O], f32, tag="ps2", name="ps2")
                        if not zero_b2:
                            nc.tensor.matmul(ps2[:], ones1[:], b2row[:],
                                             start=True, stop=False)
                        for hc in range(NH):
                            nc.tensor.matmul(
                                ps2[:], ht[hc][:, ds(bs * 128, 128)],
                                w2t[:, hc, :],
                                start=(zero_b2 and hc == 0), stop=(hc == NH - 1))
                        e_sb = e_pool.tile([128, O], bf16, tag=f"e{e}b{bs}",
                                           name=f"e{e}b{bs}")
                        nc.scalar.activation(
                            out=e_sb[:], in_=ps2[:],
                            func=mybir.ActivationFunctionType.Relu)
                        Etiles[(e, bs)] = e_sb
                        for t in range(T):
                            if PATHS[t] != 'A':
                                continue
                            if e == 0:
                                nc.vector.tensor_scalar(
                                    out=acc[t][:, bs], in0=e_sb[:],
                                    scalar1=G[bs][:, t, ds(e, 1)], scalar2=None,
                                    op0=mybir.AluOpType.mult)
                            else:
                                nc.vector.scalar_tensor_tensor(
                                    out=acc[t][:, bs], in0=e_sb[:],
                                    scalar=G[bs][:, t, ds(e, 1)],
                                    in1=acc[t][:, bs],
                                    op0=mybir.AluOpType.mult,
                                    op1=mybir.AluOpType.add)
                        if e == TE - 1:
                            for t in range(T):
                                if PATHS[t] == 'A':
                                    nc.sync.dma_start(
                                        out=out_ext[ds(bs * 128, 128), t, :],
                                        in_=acc[t][:, bs])

            # ---- fusion pass for 'P' tasks: diag(G-col) @ E in PSUM ----
            with tc.tile_pool(name="psf", bufs=6, space="PSUM") as psum_f:
                for bs in range(NBS):
                    for t in range(T):
                        if PATHS[t] != 'P':
                            continue
                        pf = psum_f.tile([128, O], f32, tag="psf", name="psf")
                        for e in range(TE):
                            dg = diag_pool.tile([128, 128], bf16, tag="diag",
                                                name="diag")
                            if e % 2 == 0:
                                col = G[bs][:, t, ds(e, 1)]
                                colb = bass.AP(
                                    tensor=col.tensor, offset=col.offset,
                                    ap=[list(col.ap[0]), [0, 128]])
                                nc.gpsimd.tensor_tensor(
                                    out=dg[:], in0=identity[:], in1=colb,
                                    op=mybir.AluOpType.mult)
                            else:
                                nc.vector.tensor_scalar(
                                    out=dg[:], in0=identity[:],
                                    scalar1=G[bs][:, t, ds(e, 1)], scalar2=None,
                                    op0=mybir.AluOpType.mult)
                            nc.tensor.matmul(
                                pf[:], dg[:], Etiles[(e, bs)][:],
                                start=(e == 0), stop=(e == TE - 1))
                        ostage = ostage_pool.tile([128, O], bf16, tag="ostage",
                                                  name="ostage")
                        nc.scalar.copy(ostage[:], pf[:])
                        nc.sync.dma_start(
                            out=out_ext[ds(bs * 128, 128), t, :],
                            in_=ostage[:])

    nc.compile()
    return nc


def _in_maps(inputs):
    import ml_dtypes

    bf = ml_dtypes.bfloat16
    x = np.ascontiguousarray(inputs["x"]).astype(bf)
    eW1 = np.ascontiguousarray(inputs["eW1"]).astype(bf)
    eW2 = np.ascontiguousarray(inputs["eW2"]).astype(bf)
    gWf = np.ascontiguousarray(
        np.asarray(inputs["gW"], np.float32).transpose(1, 0, 2)
        .reshape(D, T * TE)).astype(bf)
    eb1 = np.asarray(inputs["eb1"], np.float32)
    eb2 = np.asarray(inputs["eb2"], np.float32)
    gb = np.asarray(inputs["gb"], np.float32)
    sew = np.asarray(inputs["sew"], np.float32)
    S = np.zeros((T, TE), np.float32)
    for t in range(T):
        for e in range(E):
            S[t, t * E + e] = sew[t, e]
    gbS = np.ascontiguousarray(np.stack([gb, S]))

    zero_b1 = not np.any(eb1)
    zero_b2 = not np.any(eb2)
    zero_gb = not np.any(gb)
    common = dict(gWf=gWf, gbS=gbS, eW1=eW1, eW2=eW2)
    if not zero_b1:
        common["eb1T"] = np.ascontiguousarray(eb1.T)
    if not zero_b2:
        common["eb2"] = np.ascontiguousarray(eb2).astype(bf)
    shards = np.split(x, NCORES, axis=0)
    in_maps = [dict(common, x=np.ascontiguousarray(shards[i]))
               for i in range(NCORES)]
    return in_maps, (zero_b1, zero_b2, zero_gb)


def run_traced(trace=False, **inputs):
    from concourse.bass_utils import run_bass_kernel_spmd

    in_maps, key = _in_maps(inputs)
    if _CACHE.get("key") != key:
        _CACHE["nc"] = _build(*key)
        _CACHE["key"] = key
    res = run_bass_kernel_spmd(_CACHE["nc"], in_maps,
                               core_ids=list(range(NCORES)), trace=trace)
    out = np.concatenate([res.results[i]["out"] for i in range(NCORES)], axis=0)
    return np.asarray(out, dtype=np.float32), res


def kernel(**inputs):
    out, _ = run_traced(trace=False, **inputs)
    return out


# revision 25
# speedup vs baseline: 1.1955x; 1.1007x over previous
"""AdaTTSpUnit (16-expert MoE + 8 task gates + self-expert residual) on 8 TRN2
NeuronCores. Data-parallel over batch: each core computes 512 of the 4096 rows
end-to-end; no collectives.

Per-core math (B=512 rows, D=1024, H=512, O=512, T=8, E=2, TE=16):
  hT_e[h,b]   = relu(W1_e^T x^T + b1_e)        (PE, [h,b] layout - no transposes
  E_e[b,o]    = relu(hT_e^T W2_e + b2_e)        between layers)
  G[b,t,g]    = softmax_g(x gW + gb) + blockdiag(sew)
  out[b,t,o]  = sum_g G[b,t,g] * E_g[b,o]

Matmul operands are bf16 (f32 matmuls cost two PE passes); PSUM accumulation
is f32. The gate fusion is split by task: 'P' tasks run as diag(G-col) @ E
matmuls on the PE accumulating in PSUM (all E tiles are kept resident in
SBUF); 'A' tasks accumulate incrementally on the vector engine with
per-partition-scalar FMAs.
"""

import numpy as np

B, D, H, O = 4096, 1024, 512, 512
T, E = 8, 2
TE = T * E
NCORES = 8
BL = B // NCORES          # 512 rows per core
NBS = BL // 128           # 4 row-blocks of 128
ND = D // 128             # 8 contraction chunks for layer 1 / gates
NH = H // 128             # 4 contraction chunks for layer 2

# fusion path per (task, row-block) unit: 'P' = PE diag-matmul pass after the
# expert loop; 'A' = incremental DVE scalar_tensor_tensor
PATHS = ['P', 'P', 'P', 'P', 'P', 'A', 'A', 'A']


def _path(t, bs):
    if t == 4:
        return "A"
    if False:
        return 'A'
    return PATHS[t]

_CACHE = {}


def _build(zero_b1, zero_b2, zero_gb):
    import concourse.bass as bass
    import concourse.tile as tile
    from concourse import bacc, mybir
    from concourse.bass import ds
    from concourse.masks import make_identity

    f32 = mybir.dt.float32
    bf16 = mybir.dt.bfloat16
    nc = bacc.Bacc("TRN2", target_bir_lowering=False, debug=False,
                   num_devices=NCORES)

    x_ext = nc.dram_tensor("x", [BL, D], bf16, kind="ExternalInput").ap()
    w1_ext = nc.dram_tensor("eW1", [TE, D, H], bf16, kind="ExternalInput").ap()
    w2_ext = nc.dram_tensor("eW2", [TE, H, O], bf16, kind="ExternalInput").ap()
    gw_ext = nc.dram_tensor("gWf", [D, T * TE], bf16, kind="ExternalInput").ap()
    b1_ext = (None if zero_b1 else
              nc.dram_tensor("eb1T", [H, TE], f32, kind="ExternalInput").ap())
    b2_ext = (None if zero_b2 else
              nc.dram_tensor("eb2", [TE, O], bf16, kind="ExternalInput").ap())
    gb_ext = nc.dram_tensor("gbS", [2, T, TE], f32, kind="ExternalInput").ap()
    out_ext = nc.dram_tensor("out", [BL, T, O], bf16, kind="ExternalOutput").ap()

    with tile.TileContext(nc) as tc:
        with (
            tc.tile_pool(name="consts", bufs=1) as consts,
            tc.tile_pool(name="xin", bufs=4) as xin_pool,
            tc.tile_pool(name="w1", bufs=3) as w1_pool,
            tc.tile_pool(name="w2", bufs=3) as w2_pool,
            tc.tile_pool(name="ht", bufs=6) as ht_pool,
            tc.tile_pool(name="estore", bufs=1) as e_pool,
            tc.tile_pool(name="diag", bufs=48) as diag_pool,
            tc.tile_pool(name="ostage", bufs=4) as ostage_pool,
            tc.tile_pool(name="gsc", bufs=4) as gsc_pool,
            tc.tile_pool(name="acc", bufs=1) as acc_pool,
        ):
            identity = consts.tile([128, 128], bf16)
            make_identity(nc, identity[:])
            if not zero_b2:
                ones1 = consts.tile([1, 128], bf16)
                nc.vector.memset(ones1[:], 1.0)

            with (
                tc.tile_pool(name="pss", bufs=2, space="PSUM") as psum_small,
                tc.tile_pool(name="ps1", bufs=4, space="PSUM") as psum_l1,
                tc.tile_pool(name="ps2", bufs=2, space="PSUM") as psum_l2,
            ):
                # ---- x transpose: xT[d-part, dchunk, b] ----------------
                xT = consts.tile([128, ND, BL], bf16)
                x_ts = []
                for bs in range(NBS):
                    x_t = xin_pool.tile([128, D], bf16, tag=f"x{bs}",
                                        name=f"x{bs}")
                    xq = nc.sync if bs % 2 == 0 else nc.scalar
                    xq.dma_start(out=x_t[:], in_=x_ext[ds(bs * 128, 128), :])
                    x_ts.append(x_t)
                for dc in range(ND):
                    for bs in range(NBS):
                        tp = psum_small.tile([128, 128], bf16, tag="pss",
                                             name="tp")
                        nc.tensor.transpose(tp[:], x_ts[bs][:, ds(dc * 128, 128)],
                                            identity[:])
                        nc.vector.tensor_copy(xT[:, dc, ds(bs * 128, 128)],
                                              tp[:])

                # gate weights / bias consts (emitted after x so the
                # first expert's weight DMAs win the queue)
                gw_sb = consts.tile([128, ND, T * TE], bf16)
                nc.sync.dma_start(
                    out=gw_sb[:],
                    in_=gw_ext.rearrange("(dc dp) g -> dp dc g", dp=128))
                if not zero_b1:
                    b1_sb = consts.tile([128, NH, TE], f32)
                    for hc in range(NH):
                        nc.sync.dma_start(out=b1_sb[:, hc, :],
                                          in_=b1_ext[ds(hc * 128, 128), :])
                gbS_sb = consts.tile([128, 2, T, TE], f32)
                nc.sync.dma_start(
                    out=gbS_sb[:],
                    in_=bass.AP(tensor=gb_ext.tensor, offset=gb_ext.offset,
                                ap=[[0, 128]] + list(gb_ext.ap)),
                )
                gb_row = gbS_sb[:, 0]     # [128, T, TE]
                s_row = gbS_sb[:, 1]      # [128, T, TE]

                # ---- gates: G[b, t, g] per row-block -------------------
                G = []
                for bs in range(NBS):
                    lg = psum_small.tile([128, T * TE], f32, tag="pss",
                                         name="lg")
                    for dc in range(ND):
                        nc.tensor.matmul(
                            lg[:], xT[:, dc, ds(bs * 128, 128)], gw_sb[:, dc, :],
                            start=(dc == 0), stop=(dc == ND - 1))
                    logits = gsc_pool.tile([128, T, TE], f32, tag="logits")
                    lg3 = lg[:].rearrange("p (t g) -> p t g", t=T)
                    if zero_gb:
                        nc.scalar.copy(logits[:], lg3)
                    else:
                        nc.vector.scalar_tensor_tensor(
                            out=logits[:], in0=lg3, scalar=1.0, in1=gb_row,
                            op0=mybir.AluOpType.mult, op1=mybir.AluOpType.add)
                    nmx = gsc_pool.tile([128, T], f32, tag="nmx")
                    nc.vector.tensor_reduce(
                        out=nmx[:], in_=logits[:], axis=mybir.AxisListType.X,
                        op=mybir.AluOpType.max, negate=True)
                    ex = gsc_pool.tile([128, T, TE], f32, tag="ex")
                    for t in range(T):
                        nc.scalar.activation(
                            out=ex[:, t], in_=logits[:, t],
                            func=mybir.ActivationFunctionType.Exp,
                            bias=nmx[:, ds(t, 1)], scale=1.0)
                    sm = gsc_pool.tile([128, T], f32, tag="sm")
                    nc.vector.tensor_reduce(
                        out=sm[:], in_=ex[:], axis=mybir.AxisListType.X,
                        op=mybir.AluOpType.add)
                    rc = gsc_pool.tile([128, T], f32, tag="rc")
                    nc.vector.reciprocal(rc[:], sm[:])
                    g_bs = consts.tile([128, T, TE], f32, tag=f"g{bs}",
                                       name=f"g{bs}")
                    for t in range(T):
                        nc.vector.scalar_tensor_tensor(
                            out=g_bs[:, t], in0=ex[:, t],
                            scalar=rc[:, ds(t, 1)], in1=s_row[:, t],
                            op0=mybir.AluOpType.mult, op1=mybir.AluOpType.add)
                    G.append(g_bs)

                # ---- accumulators for 'A' tasks ------------------------
                acc = {t: acc_pool.tile([128, NBS, O], bf16, tag=f"acc{t}",
                                        name=f"acc{t}")
                       for t in range(T)
                       if any(_path(t, b) == 'A' for b in range(NBS))}

                # ---- expert loop: E[e][bs] resident in SBUF ------------
                Etiles = {}
                for e in range(TE):
                    w1t = w1_pool.tile([128, ND, H], bf16, tag="w1", name="w1t")
                    nsplit = 4 if e == 0 else 2
                    step = ND // nsplit
                    for j in range(nsplit):
                        nc.gpsimd.dma_start(
                            out=w1t[:, ds(j * step, step), :],
                            in_=w1_ext[e, ds(j * step * 128, step * 128), :]
                            .rearrange("(dc dp) h -> dp dc h", dp=128))
                    w2t = w2_pool.tile([128, NH, O], bf16, tag="w2", name="w2t")
                    wh = NH // 2
                    for j in range(2):
                        nc.gpsimd.dma_start(
                            out=w2t[:, ds(j * wh, wh), :],
                            in_=w2_ext[e, ds(j * wh * 128, wh * 128), :]
                            .rearrange("(hc hp) o -> hp hc o", hp=128))
                    if not zero_b2:
                        b2row = consts.tile([1, O], bf16, tag=f"b2_{e}",
                                            name=f"b2row{e}")
                        nc.sync.dma_start(out=b2row[:], in_=b2_ext[ds(e, 1), :])

                    ht = []
                    for hc in range(NH):
                        ps = psum_l1.tile([128, BL], f32, tag="ps1", name="ps1")
                        for dc in range(ND):
                            nc.tensor.matmul(
                                ps[:], w1t[:, dc, ds(hc * 128, 128)],
                                xT[:, dc, :],
                                start=(dc == 0), stop=(dc == ND - 1))
                        h_sb = ht_pool.tile([128, BL], bf16, tag="ht", name="ht")
                        nc.scalar.activation(
                            out=h_sb[:], in_=ps[:],
                            func=mybir.ActivationFunctionType.Relu,
                            bias=(0.0 if zero_b1 else b1_sb[:, hc, ds(e, 1)]),
                            scale=1.0)
                        ht.append(h_sb)

                    for bs in range(NBS):
                        ps2 = psum_l2.tile([128, O], f32, tag="ps2", name="ps2")
                        if not zero_b2:
                            nc.tensor.matmul(ps2[:], ones1[:], b2row[:],
                                             start=True, stop=False)
                        for hc in range(NH):
                            nc.tensor.matmul(
                                ps2[:], ht[hc][:, ds(bs * 128, 128)],
                                w2t[:, hc, :],
                                start=(zero_b2 and hc == 0), stop=(hc == NH - 1))
                        e_sb = e_pool.tile([128, O], bf16, tag=f"e{e}b{bs}",
                                           name=f"e{e}b{bs}")
                        nc.scalar.activation(
                            out=e_sb[:], in_=ps2[:],
                            func=mybir.ActivationFunctionType.Relu)
                        Etiles[(e, bs)] = e_sb
                        for t in range(T):
                            if _path(t, bs) != 'A':
                                continue
                            if e == 0:
                                nc.vector.tensor_scalar(
                                    out=acc[t][:, bs], in0=e_sb[:],
                                    scalar1=G[bs][:, t, ds(e, 1)], scalar2=None,
                                    op0=mybir.AluOpType.mult)
                            else:
                                nc.vector.scalar_tensor_tensor(
                                    out=acc[t][:, bs], in0=e_sb[:],
                                    scalar=G[bs][:, t, ds(e, 1)],
                                    in1=acc[t][:, bs],
                                    op0=mybir.AluOpType.mult,
                                    op1=mybir.AluOpType.add)
                        if e == TE - 1:
                            for t in range(T):
                                if _path(t, bs) == 'A':
                                    nc.sync.dma_start(
                                        out=out_ext[ds(bs * 128, 128), t, :],
                                        in_=acc[t][:, bs])

            # ---- fusion pass for 'P' tasks: diag(G-col) @ E in PSUM ----
            with tc.tile_pool(name="psf", bufs=8, space="PSUM") as psum_f:
                for bs in range(NBS):
                    for t in range(T):
                        if _path(t, bs) != 'P':
                            continue
                        pf = psum_f.tile([128, O], f32, tag="psf", name="psf")
                        for e in range(TE):
                            dg = diag_pool.tile([128, 128], bf16, tag="diag",
                                                name="diag")
                            if e % 2 == 0:
                                col = G[bs][:, t, ds(e, 1)]
                                colb = bass.AP(
                                    tensor=col.tensor, offset=col.offset,
                                    ap=[list(col.ap[0]), [0, 128]])
                                nc.gpsimd.tensor_tensor(
                                    out=dg[:], in0=identity[:], in1=colb,
                                    op=mybir.AluOpType.mult)
                            else:
                                nc.vector.tensor_scalar(
                                    out=dg[:], in0=identity[:],
                                    scalar1=G[bs][:, t, ds(e, 1)], scalar2=None,
                                    op0=mybir.AluOpType.mult)
                            nc.tensor.matmul(
                                pf[:], dg[:], Etiles[(e, bs)][:],
                                start=(e == 0), stop=(e == TE - 1))
                        ostage = ostage_pool.tile([128, O], bf16, tag="ostage",
                                                  name="ostage")
                        nc.scalar.copy(ostage[:], pf[:])
                        nc.sync.dma_start(
                            out=out_ext[ds(bs * 128, 128), t, :],
                            in_=ostage[:])

    nc.compile()
    return nc


def _in_maps(inputs):
    import ml_dtypes

    bf = ml_dtypes.bfloat16
    x = np.ascontiguousarray(inputs["x"]).astype(bf)
    eW1 = np.ascontiguousarray(inputs["eW1"]).astype(bf)
    eW2 = np.ascontiguousarray(inputs["eW2"]).astype(bf)
    gWf = np.ascontiguousarray(
        np.asarray(inputs["gW"], np.float32).transpose(1, 0, 2)
        .reshape(D, T * TE)).astype(bf)
    eb1 = np.asarray(inputs["eb1"], np.float32)
    eb2 = np.asarray(inputs["eb2"], np.float32)
    gb = np.asarray(inputs["gb"], np.float32)
    sew = np.asarray(inputs["sew"], np.float32)
    S = np.zeros((T, TE), np.float32)
    for t in range(T):
        for e in range(E):
            S[t, t * E + e] = sew[t, e]
    gbS = np.ascontiguousarray(np.stack([gb, S]))

    zero_b1 = not np.any(eb1)
    zero_b2 = not np.any(eb2)
    zero_gb = not np.any(gb)
    common = dict(gWf=gWf, gbS=gbS, eW1=eW1, eW2=eW2)
    if not zero_b1:
        common["eb1T"] = np.ascontiguousarray(eb1.T)
    if not zero_b2:
        common["eb2"] = np.ascontiguousarray(eb2).astype(bf)
    shards = np.split(x, NCORES, axis=0)
    in_maps = [dict(common, x=np.ascontiguousarray(shards[i]))
               for i in range(NCORES)]
    return in_maps, (zero_b1, zero_b2, zero_gb)


def run_traced(trace=False, **inputs):
    from concourse.bass_utils import run_bass_kernel_spmd

    in_maps, key = _in_maps(inputs)
    if _CACHE.get("key") != key:
        _CACHE["nc"] = _build(*key)
        _CACHE["key"] = key
    res = run_bass_kernel_spmd(_CACHE["nc"], in_maps,
                               core_ids=list(range(NCORES)), trace=trace)
    out = np.concatenate([res.results[i]["out"] for i in range(NCORES)], axis=0)
    return np.asarray(out, dtype=np.float32), res


def kernel(**inputs):
    out, _ = run_traced(trace=False, **inputs)
    return out
